# revision 16
# baseline (speedup 1.0000x reference)
"""GQA attention (B=2, LQ=LK=2048, D=2048, H=16, KV=4, dh=128) on 8 TRN2 cores.

Sharding: core = b*4 + kv (data parallel over batch, tensor parallel over
kv-head groups). Each core projects Q (its 4 heads) / K / V (its kv head),
runs attention with position bias, and computes its column-shard of the
output projection; the 4 partial outputs per batch are summed on host.

v5 design notes:
  - PE warmup matmuls at kernel start (HAM warm before the first real
    matmul); split first DMAs; K-matmuls grouped before V-matmuls per
    slab so the V stream never stalls PE waiting for wv
  - DMA queue order: wk/slab0/wv/slabs -> wq -> hqT slabs -> ebias lead ->
    wo (A2 inputs stream immediately behind hkvT)
  - phase C: all elementwise on DVE/ACT only (GpSimd big tensor ops
    contend with DVE on SBUF and double DVE latencies - measured).
    Pair multiplies [128,2048] bf16 2x on DVE; O-matmuls +2 slots behind;
    single denominator chain on DVE at [128,2048] with pair0's product
    serving as the chain seed (no copy); chain folded by accumulating
    ones-matmuls on PE; normalize via rank-1 PE broadcast + two in-place
    [128,512] DVE multiplies
  - PE filler balance: qproj t1 drained during t0 tiles, dproj(0) during
    t1 tiles; dproj(1) drains post-stream from a 6-slot PSUM pool (the
    stream pools are scoped closed) so evictions never gate the PE
Host sums the 4 kv-shard bf16 partials per batch in f32 and transposes.
"""

import numpy as np
import ml_dtypes

import concourse.bass as bass
import concourse.tile as tile
from concourse import bacc, mybir
from concourse.bass_utils import run_bass_kernel_spmd

DM = 2048      # model dim
LQ = 2048
LK = 2048
DH = 128       # head dim
H = 16         # query heads
KV = 4         # kv heads
G = H // KV    # query heads per kv head (4)
B = 2
KC = DM // 128   # contraction chunks (16)
LKC = LK // 128  # lk chunks (16)
NQT = 2          # lq tiles
LQT = LQ // NQT  # 1024
HF = 512         # half width (max moving free dim)

f32 = mybir.dt.float32
bf16 = mybir.dt.bfloat16

_BUILT = None


def _build():
    nc = bacc.Bacc()
    hqT = nc.declare_dram_parameter("hqT", [DM // 4, LQ * 4], bf16, isOutput=False)
    hkvT = nc.declare_dram_parameter("hkvT", [DM // 4, LK * 4], bf16, isOutput=False)
    wq = nc.declare_dram_parameter("wq", [128, KC * G * DH], bf16, isOutput=False)
    wk = nc.declare_dram_parameter("wk", [128, KC * DH], bf16, isOutput=False)
    wv = nc.declare_dram_parameter("wv", [128, KC * DH], bf16, isOutput=False)
    wo = nc.declare_dram_parameter("wo", [128, G * DM], bf16, isOutput=False)
    ebias = nc.declare_dram_parameter(
        "ebias", [G, NQT, LKC // 2, 128, 2 * LQT], bf16, isOutput=False
    )
    outT = nc.declare_dram_parameter("outT", [DM, LQ], bf16, isOutput=True)

    GW = G * DH  # 512, per-core q-head width

    with tile.TileContext(nc) as tc:
        with (
            tc.tile_pool(name="persist", bufs=1) as pp,
        ):
            ones_b = pp.tile([128, 1], bf16)
            nc.vector.memset(ones_b[:], 1.0)
            ones_rb = pp.tile([1, 128], bf16)
            nc.vector.memset(ones_rb[:], 1.0)
            scratch = pp.tile([128, HF], bf16)
            nc.gpsimd.memset(scratch[:], 0.0)

            kt_sb = pp.tile([128, LK], bf16)          # K^T [dh, lk]
            v_sb = pp.tile([128, LKC * DH], bf16)     # V chunks [lk%128, c*dh]
            qt_sb = pp.tile([128, G * LQ], bf16)      # Q^T per head 2MB
            ot_sb = pp.tile([128, G * LQ], bf16)      # O^T per head 2MB

            wop = tc.alloc_tile_pool(name="wob", bufs=1)
            wo_sb = wop.tile([128, G * DM], bf16)  # needed from tile k=4

            EB_LEAD = 4              # chunks of DMA lead for ebias pair tiles
            ebp = tc.alloc_tile_pool(name="ebias", bufs=3)
            eb_tiles = {}

            bslabp = tc.alloc_tile_pool(name="slabs_b", bufs=1)
            bslabs = [bslabp.tile([128, 4 * LQ], bf16, name=f"bslab{kc}")
                      for kc in range(KC // 4)]

            wp = tc.alloc_tile_pool(name="wqb", bufs=1)
            wq_sb = wp.tile([128, KC * GW], bf16)
            vtp = tc.alloc_tile_pool(name="vtb", bufs=1)
            vt_tmp = vtp.tile([128, LK], bf16)
            wkvp = tc.alloc_tile_pool(name="wkv", bufs=1)
            wk_sb = wkvp.tile([128, KC * DH], bf16)
            wv_sb = wkvp.tile([128, KC * DH], bf16)
            # wk quarter 0 covers kc2 0..3 - first K matmul gated on 128KB
            nc.sync.dma_start(wk_sb[:, 0:512], wk[:, 0:512])

            def issue_ebias(k, p):
                """DMA the ebias pair tile (lk-chunks 2p, 2p+1) of tile k."""
                t, h = k // G, k % G
                bt = ebp.tile([128, 2 * LQT], bf16, name="eb")
                nc.sync.dma_start(bt[:], ebias[h, t, p])
                eb_tiles[(k, p)] = bt

            # ---- Phase A: K^T and V^T from hkvT (V^T transposed to V by
            # the DMA xbar afterwards, off the critical path) ----
            with (
                tc.tile_pool(name="slabs", bufs=2) as slabp,
                tc.tile_pool(name="ps_a", bufs=1, space="PSUM") as psa,
            ):
                ps_kt = psa.tile([128, LK], f32)      # 4 banks
                ps_vt = psa.tile([128, LK], f32)      # 4 banks
                for kc in range(KC // 4):
                    slab = slabp.tile([128, 4 * LK], bf16)
                    if kc == 0:
                        # first quarter in [128,512] pieces (earliest start)
                        for n in range(4):
                            nc.sync.dma_start(slab[:, n * 512:(n + 1) * 512],
                                              hkvT[0:128, n * 512:(n + 1) * 512])
                        nc.sync.dma_start(wv_sb[:, 0:1024], wv[:, 0:1024])
                        for jj in range(1, 4):
                            nc.sync.dma_start(
                                slab[:, jj * LK:(jj + 1) * LK],
                                hkvT[0:128, jj * LK:(jj + 1) * LK],
                            )
                        for q in range(1, 4):
                            nc.sync.dma_start(wk_sb[:, q * 512:(q + 1) * 512],
                                              wk[:, q * 512:(q + 1) * 512])
                        nc.sync.dma_start(wv_sb[:, 1024:2048], wv[:, 1024:2048])
                        # PE warmup: ~10 dummy matmuls (~4us) so HAM is at
                        # 8/8 when wk+slab land; overwritten by K-proj.
                        for _ in range(10):
                            nc.tensor.matmul(
                                ps_kt[0:1, 0:HF], ones_b[:], scratch[:],
                                start=True, stop=True, skip_group_check=True,
                            )
                    else:
                        for jj in range(4):
                            nc.sync.dma_start(
                                slab[:, jj * LK:(jj + 1) * LK],
                                hkvT[kc * 128:(kc + 1) * 128,
                                     jj * LK:(jj + 1) * LK],
                            )
                    for j in range(4):
                        kc2 = 4 * kc + j
                        for n in range(LK // 512):
                            nc.tensor.matmul(
                                ps_kt[:, n * 512:(n + 1) * 512],
                                wk_sb[:, kc2 * DH:(kc2 + 1) * DH],
                                slab[:, j * LK + n * 512: j * LK + (n + 1) * 512],
                                start=(kc2 == 0), stop=(kc2 == KC - 1),
                                skip_group_check=(kc2 == 0),
                            )
                    for j in range(4):
                        kc2 = 4 * kc + j
                        for n in range(LK // 512):
                            nc.tensor.matmul(
                                ps_vt[:, n * 512:(n + 1) * 512],
                                wv_sb[:, kc2 * DH:(kc2 + 1) * DH],
                                slab[:, j * LK + n * 512: j * LK + (n + 1) * 512],
                                start=(kc2 == 0), stop=(kc2 == KC - 1),
                            )
                # DMA tail: A2 inputs, ebias lead, V transposes, wo
                nc.sync.dma_start(wq_sb[:], wq[:])
                for kc in range(KC // 4):
                    nc.sync.dma_start(bslabs[kc][:], hqT[kc * 128:(kc + 1) * 128, :])
                for p in range((EB_LEAD + 1) // 2):
                    issue_ebias(0, p)
                # evictions split across DVE/ACT halves to unblock A2 sooner
                nc.vector.tensor_copy(kt_sb[:, 0:1024], ps_kt[:, 0:1024])
                nc.scalar.activation(kt_sb[:, 1024:2048], ps_kt[:, 1024:2048],
                                     mybir.ActivationFunctionType.Copy)
                nc.vector.tensor_copy(vt_tmp[:, 0:1024], ps_vt[:, 0:1024])
                nc.scalar.activation(vt_tmp[:, 1024:2048], ps_vt[:, 1024:2048],
                                     mybir.ActivationFunctionType.Copy)
                # V^T -> V chunk transposes through the DMA xbar (each is
                # ~1.2us of Sync-queue occupancy - issued after the ebias
                # lead so tile 0's multiplies are never starved)
                for c in range(LKC):
                    nc.sync.dma_start_transpose(
                        v_sb[:, c * DH:(c + 1) * DH],
                        vt_tmp[:, c * 128:(c + 1) * 128],
                    )
                nc.sync.dma_start(wo_sb[:], wo[:])
            wkvp.release()
            vtp.release()

            # ---- Phase A2: Qproj for t0's four heads, kc-streamed so the
            # matmuls chase the hq slab DMAs (4x [128,1024] = 8 banks). ----
            with (
                tc.tile_pool(name="ps_q", bufs=1, space="PSUM") as psqp,
            ):
                psq = [psqp.tile([128, LQT], f32, name=f"psq{h}") for h in range(G)]

                def qmm(kc2, h, half):
                    kc, j = divmod(kc2, 4)
                    nc.tensor.matmul(
                        psq[h][:, half * HF:(half + 1) * HF],
                        wq_sb[:, kc2 * GW + h * DH: kc2 * GW + (h + 1) * DH],
                        bslabs[kc][:, j * LQ + half * HF:
                                    j * LQ + (half + 1) * HF],
                        start=(kc2 == 0), stop=(kc2 == KC - 1),
                    )

                def qevict(h):
                    if h % 2 == 0:
                        nc.scalar.activation(
                            qt_sb[:, h * LQ: h * LQ + LQT], psq[h][:],
                            mybir.ActivationFunctionType.Copy,
                        )
                    else:
                        nc.vector.tensor_copy(
                            qt_sb[:, h * LQ: h * LQ + LQT], psq[h][:]
                        )

                # slabs 0-2 kc-major (chase the bslab DMAs); last slab
                # h-outer so each head's eviction overlaps the next head's
                # matmuls instead of head-of-line blocking DVE/ACT at the
                # start of phase C
                for kc2 in range(KC - 4):
                    for h in range(G):
                        for half in range(2):
                            qmm(kc2, h, half)
                for h in range(G):
                    for kc2 in range(KC - 4, KC):
                        for half in range(2):
                            qmm(kc2, h, half)
                    qevict(h)

            # ---- Phase C: merged attention + projections stream ----
            NT = G * NQT  # 8 stream tiles, t-major: k = t*G + h
            NSLOT = NT * LKC
            state = {}
            fillers = []  # deque of closures, each emits ~1 PE instr
            laters = {}
            LA = 2            # S/exp lookahead (chunk slots)
            ED = 2            # extra slots before a pair's O-matmuls

            def pump(i, nslots):
                # drain fillers evenly over the remaining chunk slots
                rem = max(1, nslots - i - 1)
                n = min(4, max(2, -(-len(fillers) // rem)))
                for _ in range(min(n, len(fillers))):
                    fillers.pop(0)()

            def queue_dproj(t, pool, reorder=False):
                """Output projection for lq-tile t: 32 (dmt, half) groups.
                reorder: emit each group's h=3 matmul LAG groups late so the
                drain never head-of-line blocks on the last finish_tile."""
                gstates = {}

                def mk(dmt, half, h, t=t):
                    gstate = gstates.setdefault(dmt, {})

                    def run():
                        if h == 0:
                            gstate[half] = pool.tile(
                                [128, HF], f32, name="slot")
                            if half == 0:
                                gstate["out"] = outp.tile(
                                    [128, LQT], bf16, name="dout")
                        slot = gstate[half]
                        nc.tensor.matmul(
                            slot[:],
                            wo_sb[:, h * DM + dmt * 128:
                                  h * DM + (dmt + 1) * 128],
                            ot_sb[:, h * LQ + t * LQT + half * HF:
                                  h * LQ + t * LQT + (half + 1) * HF],
                            start=(h == 0), stop=(h == G - 1),
                        )
                        if h == G - 1:
                            o = gstate["out"]
                            if (dmt + half) % 2 == 0:
                                nc.vector.tensor_copy(
                                    o[:, half * HF:(half + 1) * HF],
                                    slot[:],
                                )
                            else:
                                nc.scalar.activation(
                                    o[:, half * HF:(half + 1) * HF],
                                    slot[:],
                                    mybir.ActivationFunctionType.Copy,
                                )
                            if half == 1:
                                nc.sync.dma_start(
                                    outT[dmt * 128:(dmt + 1) * 128,
                                         t * LQT:(t + 1) * LQT],
                                    o[:],
                                )
                    return run

                units = [(dmt, half) for dmt in range(DM // 128)
                         for half in range(2)]
                if not reorder:
                    for dmt, half in units:
                        for h in range(G):
                            fillers.append(mk(dmt, half, h))
                else:
                    LAG = 5
                    pend = []
                    for gi, (dmt, half) in enumerate(units):
                        for h in range(G - 1):
                            fillers.append(mk(dmt, half, h))
                        pend.append((dmt, half))
                        if gi >= LAG:
                            d2, h2 = pend.pop(0)
                            fillers.append(mk(d2, h2, G - 1))
                    for d2, h2 in pend:
                        fillers.append(mk(d2, h2, G - 1))

            with (
                tc.tile_pool(name="outb", bufs=4) as outp,
            ):
                with (
                    tc.tile_pool(name="ps_s", bufs=2, space="PSUM") as pss,
                    tc.tile_pool(name="ps_o", bufs=1, space="PSUM") as pso,
                    tc.tile_pool(name="proj", bufs=2, space="PSUM") as projp,
                    tc.tile_pool(name="ptb", bufs=2) as ptp,
                    tc.tile_pool(name="pt2b", bufs=4) as pt2p,
                    tc.tile_pool(name="accb", bufs=4) as accp,
                    tc.tile_pool(name="rcb", bufs=1) as rcp,
                    tc.tile_pool(name="bcb", bufs=1) as bcbp,
                ):
                    def s_exp(k, c):
                        """S matmuls + exp for chunk c of stream tile k."""
                        t, h = k // G, k % G
                        p, sub = divmod(c, 2)
                        ps_s = pss.tile([128, LQT], f32, name="ps_s")
                        for half in range(2):
                            nc.tensor.matmul(
                                ps_s[:, half * HF:(half + 1) * HF],
                                kt_sb[:, c * 128:(c + 1) * 128],
                                qt_sb[:, h * LQ + t * LQT + half * HF:
                                      h * LQ + t * LQT + (half + 1) * HF],
                                start=True, stop=True,
                            )
                        if sub == 0:
                            pt = ptp.tile([128, 2 * LQT], bf16, name="pt")
                            state[("pt", k, p)] = pt
                        pt = state[("pt", k, p)]
                        nc.scalar.activation(
                            pt[:, sub * LQT:(sub + 1) * LQT], ps_s[:],
                            mybir.ActivationFunctionType.Exp,
                        )

                    def mult(k, p):
                        """DVE pair multiply P^T = exp(S^T)*ebias^T [128,2048]."""
                        pt = state.pop(("pt", k, p))
                        bt = eb_tiles.pop((k, p))
                        pt2 = pt2p.tile([128, 2 * LQT], bf16, name="pt2")
                        nc.vector.tensor_tensor(
                            pt2[:], pt[:], bt[:], op=mybir.AluOpType.mult
                        )
                        return pt2

                    def mk_emit(k, p, pt2):
                        def run():
                            if p == 0:
                                state[("o", k)] = pso.tile(
                                    [128, LQT], f32, name="ps_o")
                            ps_o = state[("o", k)]
                            for sub in range(2):
                                c = 2 * p + sub
                                for half in range(2):
                                    nc.tensor.matmul(
                                        ps_o[:, half * HF:(half + 1) * HF],
                                        v_sb[:, c * DH:(c + 1) * DH],
                                        pt2[:, sub * LQT + half * HF:
                                             sub * LQT + (half + 1) * HF],
                                        start=(c == 0), stop=(c == LKC - 1),
                                    )
                        return run

                    def mk_chain(k, p, pt2):
                        def run():
                            # pair0's product seeds the chain (no copy)
                            if p == 0:
                                state[("aA", k)] = pt2
                                return
                            prev = state.pop(("aA", k))
                            acc = accp.tile([128, 2 * LQT], bf16, name="acc")
                            nc.vector.tensor_tensor(
                                acc[:], prev[:], pt2[:], op=mybir.AluOpType.add
                            )
                            state[("aA", k)] = acc
                        return run

                    def evict_o(k):
                        """O^T eviction (split ACT/DVE) frees the pso buffer."""
                        t, h = k // G, k % G
                        q_off = h * LQ + t * LQT
                        ps_o = state.pop(("o", k))
                        nc.scalar.activation(
                            ot_sb[:, q_off:q_off + HF], ps_o[:, 0:HF],
                            mybir.ActivationFunctionType.Copy,
                        )
                        nc.vector.tensor_copy(
                            ot_sb[:, q_off + HF:q_off + LQT], ps_o[:, HF:LQT],
                        )

                    def finish_tile(k, last=False):
                        """Deferred denominator + normalize for tile k. Runs
                        mid tile k+1 so no queue waits on the chain. For the
                        last tile the bf16 copies go to DVE so the drain's
                        ACT evictions are never queued behind them."""
                        t, h = k // G, k % G
                        q_off = h * LQ + t * LQT
                        acc_a = state.pop(("aA", k))
                        rc = rcp.tile([1, LQT], f32, name="rc")
                        for half in range(2):
                            ps_r = projp.tile([128, HF], f32, name="slot")
                            for idx in range(2):
                                nc.tensor.matmul(
                                    ps_r[0:1, :], ones_b[:],
                                    acc_a[:, idx * LQT + half * HF:
                                          idx * LQT + (half + 1) * HF],
                                    start=(idx == 0), stop=(idx == 1),
                                )
                            nc.vector.reciprocal_approx_fast(
                                out=rc[:, half * HF:(half + 1) * HF],
                                in_=ps_r[0:1, :],
                            )
                        rc_b = rcp.tile([1, LQT], bf16, name="rc16")
                        if last:
                            nc.vector.tensor_copy(rc_b[:], rc[:])
                        else:
                            nc.scalar.activation(
                                rc_b[:], rc[:],
                                mybir.ActivationFunctionType.Copy
                            )
                        bcb = bcbp.tile([128, LQT], bf16, name="bcb")
                        for half in range(2):
                            ps_bc = projp.tile([128, HF], f32, name="slot")
                            nc.tensor.matmul(
                                ps_bc[:], ones_rb[:],
                                rc_b[:, half * HF:(half + 1) * HF],
                                start=True, stop=True,
                            )
                            if last:
                                nc.vector.tensor_copy(
                                    bcb[:, half * HF:(half + 1) * HF],
                                    ps_bc[:],
                                )
                            else:
                                nc.scalar.activation(
                                    bcb[:, half * HF:(half + 1) * HF],
                                    ps_bc[:],
                                    mybir.ActivationFunctionType.Copy,
                                )
                        nc.vector.tensor_tensor(
                            ot_sb[:, q_off:q_off + LQT],
                            ot_sb[:, q_off:q_off + LQT],
                            bcb[:],
                            op=mybir.AluOpType.mult,
                        )

                    def queue_qproj(k):
                        """Qproj for stream tile k (t=1 heads), 2 half-jobs."""
                        t, h = k // G, k % G
                        for half in range(2):
                            hstate = {}

                            def mk(kc2, half=half, hstate=hstate, t=t, h=h):
                                def run():
                                    kc, j = divmod(kc2, 4)
                                    if kc2 == 0:
                                        hstate["slot"] = projp.tile(
                                            [128, HF], f32, name="slot")
                                    slot = hstate["slot"]
                                    nc.tensor.matmul(
                                        slot[:],
                                        wq_sb[:, kc2 * GW + h * DH:
                                              kc2 * GW + (h + 1) * DH],
                                        bslabs[kc][:, j * LQ + t * LQT + half * HF:
                                                    j * LQ + t * LQT + (half + 1) * HF],
                                        start=(kc2 == 0), stop=(kc2 == KC - 1),
                                    )
                                    if kc2 == KC - 1:
                                        if half == 0:
                                            nc.vector.tensor_copy(
                                                qt_sb[:, h * LQ + t * LQT + half * HF:
                                                      h * LQ + t * LQT + (half + 1) * HF],
                                                slot[:],
                                            )
                                        else:
                                            nc.scalar.activation(
                                                qt_sb[:, h * LQ + t * LQT + half * HF:
                                                      h * LQ + t * LQT + (half + 1) * HF],
                                                slot[:],
                                                mybir.ActivationFunctionType.Copy,
                                            )
                                return run
                            for kc2 in range(KC):
                                fillers.append(mk(kc2))

                    for h in range(G):  # Qproj fillers for t=1 (k=4..7)
                        queue_qproj(G + h)

                    flat = [(k, c) for k in range(NT) for c in range(LKC)]
                    for i in range(-LA, NSLOT):
                        # lookahead: S+exp (+ pair multiply) for slot i+LA
                        j = i + LA
                        if j < NSLOT:
                            k2, c2 = flat[j]
                            s_exp(k2, c2)
                            if c2 % 2 == 1:
                                p2 = c2 // 2
                                pt2 = mult(k2, p2)
                                laters.setdefault(j + ED, []).append(
                                    mk_emit(k2, p2, pt2))
                                laters.setdefault(j + 2, []).append(
                                    mk_chain(k2, p2, pt2))
                        if i < 0:
                            continue
                        k, c = flat[i]
                        for fn in laters.pop(i, []):
                            fn()
                        if i + EB_LEAD < NSLOT:
                            ke, ce = flat[i + EB_LEAD]
                            if ce % 2 == 0:
                                issue_ebias(ke, ce // 2)
                        if c == 8 and k > 0:
                            finish_tile(k - 1)
                            if k == G:
                                queue_dproj(0, projp)
                        pump(i, NSLOT // 2 if i < NSLOT // 2 else NSLOT)
                        if c == LKC - 1:
                            laters.setdefault(i + 2, []).append(
                                lambda k=k: evict_o(k))
                    for i in range(NSLOT, NSLOT + LA + 3):
                        for fn in laters.pop(i, []):
                            fn()
                    finish_tile(NT - 1, last=True)

                # ---- drain: dproj(1) from a deep PSUM pool so evictions
                # never gate the PE ----
                with (
                    tc.tile_pool(name="drainps", bufs=8, space="PSUM") as drainp,
                ):
                    queue_dproj(1, drainp, reorder=True)
                    while fillers:
                        pump(0, 1)
            wp.release()
            bslabp.release()
            ebp.release()
            wop.release()

    nc.finalize()
    return nc


def _get_nc():
    global _BUILT
    if _BUILT is None:
        _BUILT = _build()
    return _BUILT


def kernel(hidden_q, hidden_kv, attention_mask, position_bias, Wq, Wk, Wv, Wo,
           _trace=False):
    hidden_q = np.asarray(hidden_q, np.float32)
    hidden_kv = np.asarray(hidden_kv, np.float32)
    position_bias = np.asarray(position_bias, np.float32)
    Wq = np.asarray(Wq, np.float32)
    Wk = np.asarray(Wk, np.float32)
    Wv = np.asarray(Wv, np.float32)
    Wo = np.asarray(Wo, np.float32)
    # attention_mask is all-ones by problem spec; masking is a no-op.

    inv4 = 1.0 / np.sqrt(np.sqrt(np.float32(DH)))
    GW = G * DH

    def sb_layout(a, perm4=False):
        # [dm, w] -> [128, KC*w] with contraction chunk kc at cols [kc*w,(kc+1)*w)
        # perm4: chunk kc2=4*kc+j covers dm rows kc*512+4p+j, matching the
        # [DM//4, 4*L] packed slab view of the activations.
        w = a.shape[1]
        if perm4:
            a = a.reshape(KC // 4, 128, 4, w).transpose(0, 2, 1, 3).reshape(DM, w)
        out = np.ascontiguousarray(
            a.reshape(KC, 128, w).transpose(1, 0, 2).reshape(128, KC * w)
        )
        return np.asarray(out.astype(ml_dtypes.bfloat16))

    wq_s, wk_s, wv_s, wo_s, eb_s = [], [], [], [], []
    WqT = (Wq.T * inv4).astype(np.float32)   # [dm, H*dh]
    WkT = (Wk.T * inv4).astype(np.float32)   # [dm, KV*dh]
    WvT = Wv.T.astype(np.float32)            # [dm, KV*dh]
    for kv in range(KV):
        wq_s.append(sb_layout(np.ascontiguousarray(WqT[:, kv * GW:(kv + 1) * GW]), perm4=True))
        wk_s.append(sb_layout(np.ascontiguousarray(WkT[:, kv * DH:(kv + 1) * DH]), perm4=True))
        wv_s.append(sb_layout(np.ascontiguousarray(WvT[:, kv * DH:(kv + 1) * DH]), perm4=True))
        # wo layout: [128(dh), G*dm]; head h cols = Wo[:, kv*GW+h*DH : +DH].T
        wo_kv = Wo[:, kv * GW:(kv + 1) * GW].T  # [GW, dm]
        wo_s.append(np.asarray(np.ascontiguousarray(
            wo_kv.reshape(G, DH, DM).transpose(1, 0, 2).reshape(128, G * DM)
        ).astype(ml_dtypes.bfloat16)))
        ebT = np.exp(np.ascontiguousarray(
            position_bias[kv * G:(kv + 1) * G].transpose(0, 2, 1)
        )).astype(ml_dtypes.bfloat16)
        # -> [G, NQT, LKC//2, 128, 2*LQT]: chunk-pair tiles, 4KB DMA rows
        eb_s.append(np.ascontiguousarray(
            ebT.reshape(G, LKC // 2, 2, 128, NQT, LQT)
               .transpose(0, 4, 1, 3, 2, 5)
               .reshape(G, NQT, LKC // 2, 128, 2 * LQT)
        ))

    hqT = [np.asarray(np.ascontiguousarray(hidden_q[b].T).astype(ml_dtypes.bfloat16))
           .reshape(DM // 4, 4 * LQ) for b in range(B)]
    hkvT = [np.asarray(np.ascontiguousarray(hidden_kv[b].T).astype(ml_dtypes.bfloat16))
            .reshape(DM // 4, 4 * LK) for b in range(B)]

    in_maps = []
    for core in range(8):
        b, kv = divmod(core, KV)
        in_maps.append({
            "hqT": hqT[b], "hkvT": hkvT[b],
            "wq": wq_s[kv], "wk": wk_s[kv], "wv": wv_s[kv], "wo": wo_s[kv],
            "ebias": np.asarray(eb_s[kv]),
        })

    nc = _get_nc()
    res = run_bass_kernel_spmd(nc, in_maps, core_ids=list(range(8)), trace=_trace)
    kernel.last_exec_time_ns = res.exec_time_ns
    kernel.last_result = res

    out = np.empty((B, LQ, DM), np.float32)
    for b in range(B):
        acc = res.results[b * KV]["outT"].astype(np.float32)
        for kv in range(1, KV):
            acc += res.results[b * KV + kv]["outT"].astype(np.float32)
        out[b] = acc.T
    return out
